# revision 8
# baseline (speedup 1.0000x reference)
"""Trainium2 Bass kernel for the C2FAB_Heads problem.

Computation (see problem reference):
  C_u  = relu(gelu(LN(x_u) @ W1 + b1) @ W2 + b2)            [B,S,D]
  Phi  = concat(fwd_iir + bwd_iir for lam_fast, lam_slow)   [B,S,2D]
  R_q  = gelu(LN(x_q) @ W3 + b3) @ W4 + b4                  [B,Q,2D]

Sharding: 8 cores = (batch 4) x (seq half 2). Each core runs the MLPs on its
2048 x_u tokens + 512 x_q tokens and the 4 local IIR scans; the host applies
the exact cross-half scan carry correction during unshard.

Device layout trick: LN is folded into the first matmul. The host sends
x^T * rstd (H on partitions) plus two extra contraction rows (r*mu, ones);
the weight matrix gains matching rows (-u, c) where u = g @ W, c = b_ln @ W + b.
Then LN(x) @ W + b == xaug^T.T @ Waug, one fp32 PSUM accumulation.
"""

import numpy as np

B, S, H, HID, D, Q = 4, 4096, 4096, 256, 8, 1024
NCORES = 8
TOK_U = S // 2  # 2048 x_u tokens per core
TOK_Q = Q // 2  # 512 x_q tokens per core
CH = 512        # token chunk (matmul free dim)
NCH_U = TOK_U // CH
KA = H + 2      # augmented contraction dim
EPS = 1e-5

_CACHE = {}


def _build_nc():
    import concourse.bacc as bacc
    import concourse.mybir as mybir
    import concourse.tile as tile

    f32 = mybir.dt.float32
    AF = mybir.ActivationFunctionType
    ALU = mybir.AluOpType

    nc = bacc.Bacc(None, target_bir_lowering=False, debug=False)

    xu_aug = nc.dram_tensor("xu_aug", [KA, TOK_U], f32, kind="ExternalInput")
    xq_aug = nc.dram_tensor("xq_aug", [KA, TOK_Q], f32, kind="ExternalInput")
    w1c = nc.dram_tensor("w1c", [KA, HID], f32, kind="ExternalInput")
    w3c = nc.dram_tensor("w3c", [KA, HID], f32, kind="ExternalInput")
    w2c = nc.dram_tensor("w2c", [HID + 1, D], f32, kind="ExternalInput")
    w4c = nc.dram_tensor("w4c", [HID + 1, 2 * D], f32, kind="ExternalInput")
    lam = nc.dram_tensor("lam", [2 * D, TOK_U], f32, kind="ExternalInput")

    out_cuT = nc.dram_tensor("cuT", [D, TOK_U], f32, kind="ExternalOutput")
    out_scan = nc.dram_tensor("scan", [4 * D, TOK_U], f32, kind="ExternalOutput")
    out_rqT = nc.dram_tensor("rqT", [2 * D, TOK_Q], f32, kind="ExternalOutput")

    KG = 8                 # k-tiles per x dma group
    NG = (H // 128) // KG  # 4 groups of 8 k-tiles

    with tile.TileContext(nc) as tc:
        with (
            tc.tile_pool(name="wpool", bufs=1) as wpool,
            tc.tile_pool(name="xpool", bufs=4) as xpool,
            tc.tile_pool(name="xtail", bufs=3) as xtailp,
            tc.tile_pool(name="hpool", bufs=4) as hpool,
            tc.tile_pool(name="cpool", bufs=1) as cpool,
            tc.tile_pool(name="opool", bufs=1) as opool,
            tc.tile_pool(name="ph", bufs=4, space="PSUM") as php,
            tc.tile_pool(name="pc", bufs=2, space="PSUM") as pcp,
            tc.tile_pool(name="pq", bufs=1, space="PSUM") as pqp,
        ):
            # ---- resident weights ----
            w1sb = wpool.tile([128, H // 128, HID], f32, tag="w1sb")
            nc.sync.dma_start(
                out=w1sb, in_=w1c[0:H, :].rearrange("(k p) m -> p k m", p=128)
            )
            w1tl = wpool.tile([2, HID], f32, tag="w1tl")
            nc.sync.dma_start(out=w1tl, in_=w1c[H : H + 2, :])

            w3sb = wpool.tile([128, H // 128, HID], f32, tag="w3sb")
            nc.sync.dma_start(
                out=w3sb, in_=w3c[0:H, :].rearrange("(k p) m -> p k m", p=128)
            )
            w3tl = wpool.tile([2, HID], f32, tag="w3tl")
            nc.sync.dma_start(out=w3tl, in_=w3c[H : H + 2, :])

            w2sb = wpool.tile([128, 2, D], f32, tag="w2sb")
            nc.sync.dma_start(
                out=w2sb, in_=w2c[0:HID, :].rearrange("(k p) m -> p k m", p=128)
            )
            w2tl = wpool.tile([1, D], f32, tag="w2tl")
            nc.sync.dma_start(out=w2tl, in_=w2c[HID : HID + 1, :])

            w4sb = wpool.tile([128, 2, 2 * D], f32, tag="w4sb")
            nc.sync.dma_start(
                out=w4sb, in_=w4c[0:HID, :].rearrange("(k p) m -> p k m", p=128)
            )
            w4tl = wpool.tile([1, 2 * D], f32, tag="w4tl")
            nc.sync.dma_start(out=w4tl, in_=w4c[HID : HID + 1, :])

            lam_f = wpool.tile([D, TOK_U], f32, tag="lam_f")
            nc.sync.dma_start(out=lam_f, in_=lam[0:D, :])
            lam_s = wpool.tile([D, TOK_U], f32, tag="lam_s")
            nc.sync.dma_start(out=lam_s, in_=lam[D : 2 * D, :])

            ones_sb = wpool.tile([1, CH], f32, tag="ones_sb")
            nc.vector.memset(ones_sb, 1.0)

            cuT = cpool.tile([D, TOK_U], f32, tag="cuT")

            def mlp_chunk(xdram, col0, wsb, wtl, w2sb_, w2tl_, dout, relu, out_sl):
                """One 512-token chunk of LN+Linear+GELU+Linear(+ReLU)."""
                # load the augmented-x chunk: NG groups of KG k-tiles + tail
                xts = []
                for g in range(NG):
                    xt = xpool.tile([128, KG, CH], f32, tag="xt")
                    nc.sync.dma_start(
                        out=xt,
                        in_=xdram[g * KG * 128 : (g + 1) * KG * 128,
                                  col0 : col0 + CH].rearrange(
                            "(k p) t -> p k t", p=128
                        ),
                    )
                    xts.append(xt)
                xtl = xtailp.tile([2, CH], f32, tag="xtl")
                nc.sync.dma_start(out=xtl, in_=xdram[H : H + 2, col0 : col0 + CH])

                hts = []
                for m in range(2):
                    pht = php.tile([128, CH], f32, tag="pht")
                    for g in range(NG):
                        for j in range(KG):
                            k = g * KG + j
                            nc.tensor.matmul(
                                pht,
                                wsb[:, k, m * 128 : (m + 1) * 128],
                                xts[g][:, j, :],
                                start=(k == 0),
                                stop=False,
                            )
                    nc.tensor.matmul(
                        pht,
                        wtl[:, m * 128 : (m + 1) * 128],
                        xtl,
                        start=False,
                        stop=True,
                    )
                    ht = hpool.tile([128, CH], f32, tag="ht")
                    nc.scalar.activation(out=ht, in_=pht, func=AF.Gelu)
                    hts.append(ht)

                pct = (pcp if relu else pqp).tile(
                    [dout, CH], f32, tag=("pct" if relu else "pqt")
                )
                nc.tensor.matmul(pct, w2sb_[:, 0, :], hts[0], start=True, stop=False)
                nc.tensor.matmul(pct, w2sb_[:, 1, :], hts[1], start=False, stop=False)
                nc.tensor.matmul(pct, w2tl_, ones_sb, start=False, stop=True)
                if relu:
                    nc.scalar.activation(out=out_sl, in_=pct, func=AF.Relu)
                else:
                    nc.scalar.activation(out=out_sl, in_=pct, func=AF.Copy)

            # charge path: 4 chunks of 512 x_u tokens
            for c in range(NCH_U):
                mlp_chunk(
                    xu_aug, c * CH, w1sb, w1tl, w2sb, w2tl, D,
                    True, cuT[:, c * CH : (c + 1) * CH],
                )
            nc.sync.dma_start(out=out_cuT[:, :], in_=cuT)

            # query path: 1 chunk of 512 x_q tokens
            rqT = opool.tile([2 * D, TOK_Q], f32, tag="rqT")
            mlp_chunk(xq_aug, 0, w3sb, w3tl, w4sb, w4tl, 2 * D, False, rqT[:, :])
            nc.sync.dma_start(out=out_rqT[:, :], in_=rqT)

            # IIR scans: state = lam*state + x along the free (seq) dim.
            # backward scans use reversed views so results land in natural
            # t order.
            scan_specs = [
                (lam_f, False),  # fwd fast
                (lam_s, False),  # fwd slow
                (lam_f, True),   # bwd fast
                (lam_s, True),   # bwd slow
            ]
            for i, (lam_t, rev) in enumerate(scan_specs):
                so = opool.tile([D, TOK_U], f32, tag=f"sout{i}")
                nc.vector.tensor_tensor_scan(
                    out=so[:, ::-1] if rev else so[:, :],
                    data0=lam_t[:, :],
                    data1=cuT[:, ::-1] if rev else cuT[:, :],
                    initial=0.0, op0=ALU.mult, op1=ALU.add,
                )
                nc.sync.dma_start(
                    out=out_scan[i * D : (i + 1) * D, :], in_=so
                )

    nc.compile()
    return nc


def _host_prep(inputs):
    f32 = np.float32
    x_u = np.asarray(inputs["x_u"], f32)
    x_q = np.asarray(inputs["x_q"], f32)
    W1 = np.asarray(inputs["W1"], f32)
    b1 = np.asarray(inputs["b1"], f32)
    W2 = np.asarray(inputs["W2"], f32)
    b2 = np.asarray(inputs["b2"], f32)
    W3 = np.asarray(inputs["W3"], f32)
    b3 = np.asarray(inputs["b3"], f32)
    W4 = np.asarray(inputs["W4"], f32)
    b4 = np.asarray(inputs["b4"], f32)
    g1 = np.asarray(inputs["ln1_g"], f32)
    bl1 = np.asarray(inputs["ln1_b"], f32)
    g2 = np.asarray(inputs["ln2_g"], f32)
    bl2 = np.asarray(inputs["ln2_b"], f32)
    lf = np.clip(np.asarray(inputs["lam_fast"], f32), 1e-4, 1.0 - 1e-4)
    ls = np.clip(np.asarray(inputs["lam_slow"], f32), 1e-4, 1.0 - 1e-4)

    w1c = np.empty((KA, HID), f32)
    w1c[:H] = W1 * g1[:, None]
    w1c[H] = -(g1 @ W1)
    w1c[H + 1] = bl1 @ W1 + b1
    w3c = np.empty((KA, HID), f32)
    w3c[:H] = W3 * g2[:, None]
    w3c[H] = -(g2 @ W3)
    w3c[H + 1] = bl2 @ W3 + b3
    w2c = np.concatenate([W2, b2[None]], 0).astype(f32)
    w4c = np.concatenate([W4, b4[None]], 0).astype(f32)

    lam_tile = np.empty((2 * D, TOK_U), f32)
    lam_tile[:D] = lf[:, None]
    lam_tile[D:] = ls[:, None]

    def aug(x_sl):  # [T, H] -> [H+2, T]
        mu = x_sl.mean(-1, keepdims=True)
        d = x_sl - mu
        var = np.mean(d * d, -1, keepdims=True)
        r = 1.0 / np.sqrt(var + EPS)
        out = np.empty((KA, x_sl.shape[0]), f32)
        out[:H] = np.ascontiguousarray((x_sl * r).T)
        out[H] = (r * mu)[:, 0]
        out[H + 1] = 1.0
        return out

    in_maps = []
    for c in range(NCORES):
        b, hf = divmod(c, 2)
        in_maps.append(
            {
                "xu_aug": aug(x_u[b, hf * TOK_U : (hf + 1) * TOK_U]),
                "xq_aug": aug(x_q[b, hf * TOK_Q : (hf + 1) * TOK_Q]),
                "w1c": w1c, "w2c": w2c, "w3c": w3c, "w4c": w4c,
                "lam": lam_tile,
            }
        )
    return in_maps, lf, ls


def _host_post(results, lf, ls):
    f32 = np.float32
    C_u = np.empty((B, S, D), f32)
    R_q = np.empty((B, Q, 2 * D), f32)
    Phi = np.empty((B, S, 2 * D), f32)

    # lam^(i+1) for i in 0..TOK_U-1, per channel: [TOK_U, D]
    i1 = np.arange(1, TOK_U + 1, dtype=np.float64)[:, None]
    Pf = (np.asarray(lf, np.float64)[None, :] ** i1).astype(f32)
    Ps = (np.asarray(ls, np.float64)[None, :] ** i1).astype(f32)

    for b in range(B):
        c0, c1 = 2 * b, 2 * b + 1
        C_u[b, :TOK_U] = results[c0]["cuT"].T
        C_u[b, TOK_U:] = results[c1]["cuT"].T
        R_q[b, :TOK_Q] = results[c0]["rqT"].T
        R_q[b, TOK_Q:] = results[c1]["rqT"].T

        s0 = results[c0]["scan"]  # [32, 2048] rows: ff, fs, bf, bs
        s1 = results[c1]["scan"]

        # forward scans: half0 is globally correct; half1 needs carry fix
        yff = np.concatenate(
            [s0[0:D].T, s1[0:D].T + Pf * s0[0:D, -1][None, :]], 0
        )
        yfs = np.concatenate(
            [s0[D : 2 * D].T, s1[D : 2 * D].T + Ps * s0[D : 2 * D, -1][None, :]], 0
        )
        # backward scans: half1 is globally correct; half0 needs carry fix
        # fix at local t: lam^(TOK_U-t) * z[first of half1] = P[::-1] * carry
        zbf = np.concatenate(
            [s0[2 * D : 3 * D].T + Pf[::-1] * s1[2 * D : 3 * D, 0][None, :],
             s1[2 * D : 3 * D].T], 0
        )
        zbs = np.concatenate(
            [s0[3 * D : 4 * D].T + Ps[::-1] * s1[3 * D : 4 * D, 0][None, :],
             s1[3 * D : 4 * D].T], 0
        )
        Phi[b, :, :D] = yff + zbf
        Phi[b, :, D:] = yfs + zbs

    return Phi, R_q, C_u


def kernel(**inputs):
    from concourse.bass_utils import run_bass_kernel_spmd

    in_maps, lf, ls = _host_prep(inputs)
    if "nc" not in _CACHE:
        _CACHE["nc"] = _build_nc()
    nc = _CACHE["nc"]
    _CACHE["in_maps"] = in_maps
    res = run_bass_kernel_spmd(nc, in_maps, core_ids=list(range(NCORES)))
    return _host_post(res.results, lf, ls)


# revision 13
# speedup vs baseline: 28.7398x; 28.7398x over previous
"""Trainium2 Bass kernel for the C2FAB_Heads problem.

Computation (see problem reference):
  C_u  = relu(gelu(LN(x_u) @ W1 + b1) @ W2 + b2)            [B,S,D]
  Phi  = concat(fwd_iir + bwd_iir for lam_fast, lam_slow)   [B,S,2D]
  R_q  = gelu(LN(x_q) @ W3 + b3) @ W4 + b4                  [B,Q,2D]

Sharding: 8 cores = (batch 4) x (seq half 2). Each core runs the MLPs on its
2048 x_u tokens + 512 x_q tokens and the 4 local IIR scans; the host applies
the exact cross-half scan carry correction during unshard.

Device layout trick: LN is folded into the first matmul. The host sends
x^T * rstd (H on partitions) plus two extra contraction rows (r*mu, ones);
the weight matrix gains matching rows (-u, c) where u = g @ W, c = b_ln @ W + b.
Then LN(x) @ W + b == xaug^T.T @ Waug, one fp32 PSUM accumulation.
"""

import numpy as np

B, S, H, HID, D, Q = 4, 4096, 4096, 256, 8, 1024
NCORES = 8
TOK_U = S // 2  # 2048 x_u tokens per core
TOK_Q = Q // 2  # 512 x_q tokens per core
CH = 512        # token chunk (matmul free dim)
NCH_U = TOK_U // CH
KA = H + 2      # augmented contraction dim
EPS = 1e-5

_CACHE = {}


def _build_nc(reps=1):
    import concourse.bacc as bacc
    import concourse.mybir as mybir
    import concourse.tile as tile

    f32 = mybir.dt.float32
    f32r = mybir.dt.float32r
    AF = mybir.ActivationFunctionType
    ALU = mybir.AluOpType

    nc = bacc.Bacc(None, target_bir_lowering=False, debug=False)

    xu_aug = nc.dram_tensor("xu_aug", [KA, TOK_U], f32r, kind="ExternalInput")
    xq_aug = nc.dram_tensor("xq_aug", [KA, TOK_Q], f32r, kind="ExternalInput")
    w1c = nc.dram_tensor("w1c", [KA, HID], f32r, kind="ExternalInput")
    w3c = nc.dram_tensor("w3c", [KA, HID], f32r, kind="ExternalInput")
    w2c = nc.dram_tensor("w2c", [HID + 1, D], f32r, kind="ExternalInput")
    w4c = nc.dram_tensor("w4c", [HID + 1, 2 * D], f32r, kind="ExternalInput")
    lam = nc.dram_tensor("lam", [2 * D, TOK_U], f32, kind="ExternalInput")
    onesd = nc.dram_tensor("onesd", [1, CH], f32r, kind="ExternalInput")

    out_cuT = nc.dram_tensor("cuT", [D, TOK_U], f32, kind="ExternalOutput")
    out_scan = nc.dram_tensor("scan", [4 * D, TOK_U], f32, kind="ExternalOutput")
    out_rqT = nc.dram_tensor("rqT", [2 * D, TOK_Q], f32, kind="ExternalOutput")

    KG = 8                 # k-tiles per x dma group
    NG = (H // 128) // KG  # 4 groups of 8 k-tiles

    with tile.TileContext(nc) as tc:
        with (
            tc.tile_pool(name="wpool", bufs=1) as wpool,
            tc.tile_pool(name="xpool", bufs=4) as xpool,
            tc.tile_pool(name="xtail", bufs=3) as xtailp,
            tc.tile_pool(name="hpool", bufs=4) as hpool,
            tc.tile_pool(name="cpool", bufs=1) as cpool,
            tc.tile_pool(name="opool", bufs=1) as opool,
            tc.tile_pool(name="ph", bufs=4, space="PSUM") as php,
            tc.tile_pool(name="pc", bufs=2, space="PSUM") as pcp,
            tc.tile_pool(name="pq", bufs=1, space="PSUM") as pqp,
        ):
            # ---- resident weights ----
            w1sb = wpool.tile([128, H // 128, HID], f32r, tag="w1sb")
            nc.sync.dma_start(
                out=w1sb, in_=w1c[0:H, :].rearrange("(k p) m -> p k m", p=128)
            )
            w1tl = wpool.tile([2, HID], f32r, tag="w1tl")
            nc.sync.dma_start(out=w1tl, in_=w1c[H : H + 2, :])

            w3sb = wpool.tile([128, H // 128, HID], f32r, tag="w3sb")
            nc.sync.dma_start(
                out=w3sb, in_=w3c[0:H, :].rearrange("(k p) m -> p k m", p=128)
            )
            w3tl = wpool.tile([2, HID], f32r, tag="w3tl")
            nc.sync.dma_start(out=w3tl, in_=w3c[H : H + 2, :])

            w2sb = wpool.tile([128, 2, D], f32r, tag="w2sb")
            nc.sync.dma_start(
                out=w2sb, in_=w2c[0:HID, :].rearrange("(k p) m -> p k m", p=128)
            )
            w2tl = wpool.tile([1, D], f32r, tag="w2tl")
            nc.sync.dma_start(out=w2tl, in_=w2c[HID : HID + 1, :])

            w4sb = wpool.tile([128, 2, 2 * D], f32r, tag="w4sb")
            nc.sync.dma_start(
                out=w4sb, in_=w4c[0:HID, :].rearrange("(k p) m -> p k m", p=128)
            )
            w4tl = wpool.tile([1, 2 * D], f32r, tag="w4tl")
            nc.sync.dma_start(out=w4tl, in_=w4c[HID : HID + 1, :])

            lam_f = wpool.tile([D, TOK_U], f32, tag="lam_f")
            nc.sync.dma_start(out=lam_f, in_=lam[0:D, :])
            lam_s = wpool.tile([D, TOK_U], f32, tag="lam_s")
            nc.sync.dma_start(out=lam_s, in_=lam[D : 2 * D, :])

            ones_sb = wpool.tile([1, CH], f32r, tag="ones_sb")
            nc.sync.dma_start(out=ones_sb, in_=onesd[:, :])

            def mlp_chunk(xdram, col0, wsb, wtl, w2sb_, w2tl_, dout, relu, out_sl):
                """One 512-token chunk of LN+Linear+GELU+Linear(+ReLU)."""
                # load the augmented-x chunk: NG groups of KG k-tiles + tail
                xts = []
                for g in range(NG):
                    xt = xpool.tile([128, KG, CH], f32r, tag="xt")
                    nc.sync.dma_start(
                        out=xt,
                        in_=xdram[g * KG * 128 : (g + 1) * KG * 128,
                                  col0 : col0 + CH].rearrange(
                            "(k p) t -> p k t", p=128
                        ),
                    )
                    xts.append(xt)
                xtl = xtailp.tile([2, CH], f32r, tag="xtl")
                nc.sync.dma_start(out=xtl, in_=xdram[H : H + 2, col0 : col0 + CH])

                hts = []
                for m in range(2):
                    pht = php.tile([128, CH], f32, tag="pht")
                    for g in range(NG):
                        for j in range(KG):
                            k = g * KG + j
                            nc.tensor.matmul(
                                pht,
                                wsb[:, k, m * 128 : (m + 1) * 128],
                                xts[g][:, j, :],
                                start=(k == 0),
                                stop=False,
                            )
                    nc.tensor.matmul(
                        pht,
                        wtl[:, m * 128 : (m + 1) * 128],
                        xtl[:, :],
                        start=False,
                        stop=True,
                    )
                    ht = hpool.tile([128, CH], f32r, tag="ht")
                    nc.scalar.activation(out=ht, in_=pht, func=AF.Gelu)
                    hts.append(ht)

                pct = (pcp if relu else pqp).tile(
                    [dout, CH], f32, tag=("pct" if relu else "pqt")
                )
                nc.tensor.matmul(pct, w2sb_[:, 0, :],
                                 hts[0][:, :], start=True,
                                 stop=False)
                nc.tensor.matmul(pct, w2sb_[:, 1, :],
                                 hts[1][:, :], start=False,
                                 stop=False)
                nc.tensor.matmul(pct, w2tl_[:, :],
                                 ones_sb[:, :], start=False,
                                 stop=True)
                if relu:
                    nc.scalar.activation(out=out_sl, in_=pct, func=AF.Relu)
                else:
                    nc.scalar.activation(out=out_sl, in_=pct, func=AF.Copy)

            for _rep in range(reps):
                cuT = cpool.tile([D, TOK_U], f32, tag="cuT")

                # charge path: 4 chunks of 512 x_u tokens
                for c in range(NCH_U):
                    mlp_chunk(
                        xu_aug, c * CH, w1sb, w1tl, w2sb, w2tl, D,
                        True, cuT[:, c * CH : (c + 1) * CH],
                    )
                nc.sync.dma_start(out=out_cuT[:, :], in_=cuT)

                # query path: 1 chunk of 512 x_q tokens
                rqT = opool.tile([2 * D, TOK_Q], f32, tag="rqT")
                mlp_chunk(xq_aug, 0, w3sb, w3tl, w4sb, w4tl, 2 * D, False,
                          rqT[:, :])
                nc.sync.dma_start(out=out_rqT[:, :], in_=rqT)

                # IIR scans: state = lam*state + x along the free (seq) dim.
                # backward scans use reversed views so results land in
                # natural t order.
                scan_specs = [
                    (lam_f, False),  # fwd fast
                    (lam_s, False),  # fwd slow
                    (lam_f, True),   # bwd fast
                    (lam_s, True),   # bwd slow
                ]
                for i, (lam_t, rev) in enumerate(scan_specs):
                    so = opool.tile([D, TOK_U], f32, tag=f"sout{i}")
                    nc.vector.tensor_tensor_scan(
                        out=so[:, ::-1] if rev else so[:, :],
                        data0=lam_t[:, :],
                        data1=cuT[:, ::-1] if rev else cuT[:, :],
                        initial=0.0, op0=ALU.mult, op1=ALU.add,
                    )
                    nc.sync.dma_start(
                        out=out_scan[i * D : (i + 1) * D, :], in_=so
                    )

    nc.compile()
    return nc


def _host_prep(inputs):
    f32 = np.float32
    x_u = np.asarray(inputs["x_u"], f32)
    x_q = np.asarray(inputs["x_q"], f32)
    W1 = np.asarray(inputs["W1"], f32)
    b1 = np.asarray(inputs["b1"], f32)
    W2 = np.asarray(inputs["W2"], f32)
    b2 = np.asarray(inputs["b2"], f32)
    W3 = np.asarray(inputs["W3"], f32)
    b3 = np.asarray(inputs["b3"], f32)
    W4 = np.asarray(inputs["W4"], f32)
    b4 = np.asarray(inputs["b4"], f32)
    g1 = np.asarray(inputs["ln1_g"], f32)
    bl1 = np.asarray(inputs["ln1_b"], f32)
    g2 = np.asarray(inputs["ln2_g"], f32)
    bl2 = np.asarray(inputs["ln2_b"], f32)
    lf = np.clip(np.asarray(inputs["lam_fast"], f32), 1e-4, 1.0 - 1e-4)
    ls = np.clip(np.asarray(inputs["lam_slow"], f32), 1e-4, 1.0 - 1e-4)

    w1c = np.empty((KA, HID), f32)
    w1c[:H] = W1 * g1[:, None]
    w1c[H] = -(g1 @ W1)
    w1c[H + 1] = bl1 @ W1 + b1
    w3c = np.empty((KA, HID), f32)
    w3c[:H] = W3 * g2[:, None]
    w3c[H] = -(g2 @ W3)
    w3c[H + 1] = bl2 @ W3 + b3
    w2c = np.concatenate([W2, b2[None]], 0).astype(f32)
    w4c = np.concatenate([W4, b4[None]], 0).astype(f32)

    lam_tile = np.empty((2 * D, TOK_U), f32)
    lam_tile[:D] = lf[:, None]
    lam_tile[D:] = ls[:, None]

    def aug(x_sl):  # [T, H] -> [H+2, T]
        mu = x_sl.mean(-1, keepdims=True)
        d = x_sl - mu
        var = np.mean(d * d, -1, keepdims=True)
        r = 1.0 / np.sqrt(var + EPS)
        out = np.empty((KA, x_sl.shape[0]), f32)
        out[:H] = np.ascontiguousarray((x_sl * r).T)
        out[H] = (r * mu)[:, 0]
        out[H + 1] = 1.0
        return out

    in_maps = []
    for c in range(NCORES):
        b, hf = divmod(c, 2)
        in_maps.append(
            {
                "xu_aug": aug(x_u[b, hf * TOK_U : (hf + 1) * TOK_U]),
                "xq_aug": aug(x_q[b, hf * TOK_Q : (hf + 1) * TOK_Q]),
                "w1c": w1c, "w2c": w2c, "w3c": w3c, "w4c": w4c,
                "lam": lam_tile, "onesd": np.ones((1, CH), f32),
            }
        )
    return in_maps, lf, ls


def _host_post(results, lf, ls):
    f32 = np.float32
    C_u = np.empty((B, S, D), f32)
    R_q = np.empty((B, Q, 2 * D), f32)
    Phi = np.empty((B, S, 2 * D), f32)

    # lam^(i+1) for i in 0..TOK_U-1, per channel: [TOK_U, D]
    i1 = np.arange(1, TOK_U + 1, dtype=np.float64)[:, None]
    Pf = (np.asarray(lf, np.float64)[None, :] ** i1).astype(f32)
    Ps = (np.asarray(ls, np.float64)[None, :] ** i1).astype(f32)

    for b in range(B):
        c0, c1 = 2 * b, 2 * b + 1
        C_u[b, :TOK_U] = results[c0]["cuT"].T
        C_u[b, TOK_U:] = results[c1]["cuT"].T
        R_q[b, :TOK_Q] = results[c0]["rqT"].T
        R_q[b, TOK_Q:] = results[c1]["rqT"].T

        s0 = results[c0]["scan"]  # [32, 2048] rows: ff, fs, bf, bs
        s1 = results[c1]["scan"]

        # forward scans: half0 is globally correct; half1 needs carry fix
        yff = np.concatenate(
            [s0[0:D].T, s1[0:D].T + Pf * s0[0:D, -1][None, :]], 0
        )
        yfs = np.concatenate(
            [s0[D : 2 * D].T, s1[D : 2 * D].T + Ps * s0[D : 2 * D, -1][None, :]], 0
        )
        # backward scans: half1 is globally correct; half0 needs carry fix
        # fix at local t: lam^(TOK_U-t) * z[first of half1] = P[::-1] * carry
        zbf = np.concatenate(
            [s0[2 * D : 3 * D].T + Pf[::-1] * s1[2 * D : 3 * D, 0][None, :],
             s1[2 * D : 3 * D].T], 0
        )
        zbs = np.concatenate(
            [s0[3 * D : 4 * D].T + Ps[::-1] * s1[3 * D : 4 * D, 0][None, :],
             s1[3 * D : 4 * D].T], 0
        )
        Phi[b, :, :D] = yff + zbf
        Phi[b, :, D:] = yfs + zbs

    return Phi, R_q, C_u


def kernel(**inputs):
    from concourse.bass_utils import run_bass_kernel_spmd

    in_maps, lf, ls = _host_prep(inputs)
    if "nc" not in _CACHE:
        _CACHE["nc"] = _build_nc()
    nc = _CACHE["nc"]
    _CACHE["in_maps"] = in_maps
    res = run_bass_kernel_spmd(nc, in_maps, core_ids=list(range(NCORES)))
    return _host_post(res.results, lf, ls)
